# revision 4
# baseline (speedup 1.0000x reference)
import numpy as np

SQ2 = 2.0 ** 0.5
H = W = 512
HH = 256  # H//2
NCH = 8
NCORES = 8

_cache = {}


def _filters():
    hh = np.array([0.037828455506995, -0.02384946501938, -0.11062440441842, 0.37740285561265], np.float32)
    h = np.concatenate([hh, [np.float32(0.8526986790094)], hh[::-1]]).astype(np.float32)
    gg = np.array([-0.064538882628938, -0.040689417609558, 0.41809227322221], np.float32)
    g = np.concatenate([gg, [np.float32(0.78848561640566)], gg[::-1]]).astype(np.float32)
    v = np.array([0.63, -0.193, 0.0972, -0.0526, 0.0272, -0.0144], np.float32)
    f = np.concatenate([v[::-1], v]).astype(np.float32)
    f[::2] = -f[::2]
    return h, g, f


def _host_mats(h, g, f):
    # BhP [520,256]: fused 9-tap h conv + 2x avgpool (rows): out[i] = 0.5*(L[2i]+L[2i+1])
    BhP = np.zeros((520, 256), np.float32)
    for i in range(256):
        for p in (0, 1):
            r = 2 * i + p
            for u in range(9):
                BhP[r + u, i] += 0.5 * h[u]
    # GU [260,512]: upsample cA (pad2) + 7-tap g conv: Mrow[r] = sum_u g[u] D[r+u-3], D[2a']=cA[a']
    GU = np.zeros((260, 512), np.float32)
    for r in range(512):
        for u in range(7):
            al = r + u - 3
            if al % 2 == 0:
                a = al // 2 + 2  # cAp row (pad 2)
                GU[a, r] += g[u]
    GU8 = (8.0 * GU).astype(np.float32)
    # Bf256 [267,256], Bf512 [523,512]: 12-tap f bands
    Bf256 = np.zeros((267, 256), np.float32)
    for o in range(256):
        for u in range(12):
            Bf256[o + u, o] = f[u]
    Bf512 = np.zeros((523, 512), np.float32)
    for o in range(512):
        for u in range(12):
            Bf512[o + u, o] = f[u]
    ident = np.eye(128, dtype=np.float32)
    return {"BhP": BhP, "GU8": GU8, "GU": GU, "Bf256": Bf256, "Bf512": Bf512, "ident": ident}


def _build_nc():
    import concourse.bass as bass
    import concourse.bacc as bacc
    import concourse.mybir as mybir
    from concourse import tile

    FP = mybir.dt.float32
    nc = bacc.Bacc("TRN2", target_bir_lowering=False, debug=False, num_devices=NCORES)
    AP = bass.AP

    x_h = nc.dram_tensor("x", [NCH, H, W], FP, kind="ExternalInput")
    mat_hs = {}
    for nm, shp in [("BhP", (520, 256)), ("GU8", (260, 512)), ("GU", (260, 512)),
                    ("Bf256", (267, 256)), ("Bf512", (523, 512)), ("ident", (128, 128))]:
        mat_hs[nm] = nc.dram_tensor(nm, list(shp), FP, kind="ExternalInput")
    outs = {}
    for nm in ["out_c", "out_e1lo", "out_e0lo", "out_e1hi", "out_e0hi"]:
        outs[nm] = nc.dram_tensor(nm, [NCH, HH, HH], FP, kind="ExternalOutput")

    # internal DRAM
    WET = 832
    Xd = nc.dram_tensor("Xd", [H, W], FP, kind="Internal")
    cAd = nc.dram_tensor("cAd", [HH, HH], FP, kind="Internal")
    Md = nc.dram_tensor("Md", [H, W], FP, kind="Internal")
    ET2 = nc.dram_tensor("ET2", [WET, WET], FP, kind="Internal")       # DsumT tiled, origin 20,20
    EXX = nc.dram_tensor("EXX", [560, WET], FP, kind="Internal")       # XXs qext, row A+16, col B+280
    E1 = nc.dram_tensor("E1", [512, 512], FP, kind="Internal")         # S1 qext: vstack(rot256(S1), S1)
    E2 = nc.dram_tensor("E2", [512, 512], FP, kind="Internal")
    X0d = nc.dram_tensor("X0d", [HH, W], FP, kind="Internal")
    E0d = nc.dram_tensor("E0d", [HH, HH], FP, kind="Internal")

    def dram_ap(hd, off, dims):
        return AP(hd, off, [list(d) for d in dims])

    with tile.TileContext(nc) as tc:
        import contextlib
        ctx = contextlib.ExitStack()
        with ctx:
            cpool = ctx.enter_context(tc.tile_pool(name="consts", bufs=1))
            work = ctx.enter_context(tc.tile_pool(name="work", bufs=2))
            keep = ctx.enter_context(tc.tile_pool(name="keep", bufs=1))
            tmp = ctx.enter_context(tc.tile_pool(name="tmp", bufs=1))
            psum = ctx.enter_context(tc.tile_pool(name="ps", bufs=4, space="PSUM"))

            # ---- load const matrices as K-chunked [<=128, M] tiles ----
            mats = {}
            for nm, (K, M) in [("BhP", (520, 256)), ("GU8", (260, 512)), ("GU", (260, 512)),
                               ("Bf256", (267, 256)), ("Bf512", (523, 512))]:
                tl = []
                for k0 in range(0, K, 128):
                    kk = min(128, K - k0)
                    t = cpool.tile([kk, M], FP, tag=f"m_{nm}_{k0}")
                    nc.sync.dma_start(t[:], mat_hs[nm].ap()[k0:k0 + kk, :])
                    tl.append((k0, kk, t))
                mats[nm] = tl
            identt = cpool.tile([128, 128], FP, tag="ident")
            nc.sync.dma_start(identt[:], mat_hs["ident"].ap()[:, :])

            def transpose_tiles(src_tiles, R, C, pool, tag):
                # src_tiles: list (r0, nr, tile[nr, C]) covering [R, C] -> returns tiles of [C, R]
                outt = []
                for c0 in range(0, C, 128):
                    cw = min(128, C - c0)
                    t = pool.tile([cw, R], FP, tag=f"{tag}_{c0}")
                    for (r0, nr, st) in src_tiles:
                        ps = psum.tile([cw, nr], FP, tag="ps")
                        nc.tensor.transpose(ps[:, :], st[:, c0:c0 + cw], identt[:nr, :nr])
                        nc.vector.tensor_copy(t[:, r0:r0 + nr], ps[:, :])
                    outt.append((c0, cw, t))
                return outt

            def band_pass(in_tiles, mat_tiles, M, N, pool, tag):
                # out[m, n] = sum_k mat[k, m] * in[k, n]; in_tiles chunked at 128 rows
                outt = []
                for m0 in range(0, M, 128):
                    mw = min(128, M - m0)
                    t = pool.tile([mw, N], FP, tag=f"{tag}_{m0}")
                    for n0 in range(0, N, 512):
                        nw = min(512, N - n0)
                        ps = psum.tile([mw, nw], FP, tag="ps")
                        nk = len(in_tiles)
                        for ki, ((k0, kk, it), (mk0, mkk, mt)) in enumerate(zip(in_tiles, mat_tiles)):
                            assert k0 == mk0 and kk == mkk
                            nc.tensor.matmul(ps[:, :], mt[:, m0:m0 + mw], it[:, n0:n0 + nw],
                                             start=(ki == 0), stop=(ki == nk - 1))
                        nc.vector.tensor_copy(t[:, n0:n0 + nw], ps[:, :])
                    outt.append((m0, mw, t))
                return outt

            def conv2(in_tiles, Rp, Cp, mrow, mcol, Mr, Mc, pool, tag):
                # in_tiles cover padded [Rp, Cp]; returns result [Mr, Mc] tiles (normal orientation)
                p1 = band_pass(in_tiles, mats[mrow], Mr, Cp, tmp, "cvp1")
                p1t = transpose_tiles(p1, Mr, Cp, tmp, "cvt1")
                p2 = band_pass(p1t, mats[mcol], Mc, Mr, tmp, "cvp2")  # [Mc, Mr]
                return transpose_tiles(p2, Mc, Mr, pool, tag)

            def pad_per_from_dram(hd, R, C, ru, cl, Rp, Cp, pool, tag, qper=False):
                # build padded tiles [Rp, Cp] from DRAM map [R, C]; padded (k,c) -> src row (k-ru)%R col (c-cl)%C
                # qper: rows outside [0,R) additionally rotate cols by C//2
                tiles = []
                for k0 in range(0, Rp, 128):
                    kk = min(128, Rp - k0)
                    t = tmp.tile([kk, Cp], FP, tag=f"pad_{k0}")
                    # split rows into runs with same wrap-band
                    k = k0
                    while k < k0 + kk:
                        a = k - ru  # source row index (unwrapped)
                        band = 0 if 0 <= a < R else (-1 if a < 0 else 1)
                        # run length until band changes or tile ends
                        if band == -1:
                            run = min(k0 + kk - k, -a)
                        elif band == 0:
                            run = min(k0 + kk - k, R - a)
                        else:
                            run = k0 + kk - k
                        sr = a % R
                        rot = (C // 2) if (qper and band != 0) else 0
                        # cols: padded c -> src (c - cl + rot) % C ; emit contiguous segments
                        c = 0
                        while c < Cp:
                            sc = (c - cl + rot) % C
                            seg = min(Cp - c, C - sc)
                            nc.sync.dma_start(
                                t[k - k0:k - k0 + run, c:c + seg],
                                dram_ap(hd, sr * C + sc, [[C, run], [1, seg]]))
                            c += seg
                        k += run
                    tiles.append((k0, kk, t))
                return tiles

            # ================= stage 1: X = channel sum of x =================
            for r0 in range(0, H, 128):
                xa = work.tile([128, W], FP, tag="xsumA")
                xb = work.tile([128, W], FP, tag="xsumB")
                acc = work.tile([128, W], FP, tag="xsumAcc")
                nc.sync.dma_start(xa[:], x_h.ap()[0, r0:r0 + 128, :])
                nc.sync.dma_start(xb[:], x_h.ap()[1, r0:r0 + 128, :])
                nc.vector.scalar_tensor_tensor(acc[:], xa[:], 1.0, xb[:],
                                               mybir.AluOpType.mult, mybir.AluOpType.add)
                for ch in range(2, NCH):
                    xc = work.tile([128, W], FP, tag="xsumC")
                    nc.sync.dma_start(xc[:], x_h.ap()[ch, r0:r0 + 128, :])
                    nc.vector.scalar_tensor_tensor(acc[:], xc[:], 1.0, acc[:],
                                                   mybir.AluOpType.mult, mybir.AluOpType.add)
                nc.sync.dma_start(Xd.ap()[r0:r0 + 128, :], acc[:])

            # ================= stage 2: cA = pool(conv_h(X)) =================
            Xp = pad_per_from_dram(Xd, H, W, 4, 4, 520, 520, work, "Xp")
            cAt = conv2(Xp, 520, 520, "BhP", "BhP", 256, 256, keep, "cA")
            for (r0, nr, t) in cAt:
                nc.sync.dma_start(cAd.ap()[r0:r0 + nr, :], t[:])
                for ch in range(NCH):
                    nc.sync.dma_start(outs["out_c"].ap()[ch, r0:r0 + nr, :], t[:])

            # ================= stage 3: M = 8*conv_g(dup(cA)) =================
            cAp = pad_per_from_dram(cAd, HH, HH, 2, 2, 260, 260, work, "cAp")
            Mt = conv2(cAp, 260, 260, "GU8", "GU", H, W, keep, "M")
            for (r0, nr, t) in Mt:
                nc.sync.dma_start(Md.ap()[r0:r0 + nr, :], t[:])

            # ================= stage 4: Dsum = X - 8M; DsumT -> ET2 =================
            Dst = []
            for (r0, nr, mt) in Mt:
                xt = work.tile([128, W], FP, tag="Xrd")
                nc.sync.dma_start(xt[:], Xd.ap()[r0:r0 + nr, :])
                d = tmp.tile([128, W], FP, tag=f"Ds_{r0}")
                nc.vector.scalar_tensor_tensor(d[:], mt[:], -8.0, xt[:],
                                               mybir.AluOpType.mult, mybir.AluOpType.add)
                Dst.append((r0, nr, d))
            DsT = transpose_tiles(Dst, H, W, tmp, "DsT")
            # ET2[u,v] = DsumT[(u-20)%512, (v-20)%512], [832,832]
            for (r0, nr, t) in DsT:
                for (du, su, cnt_u) in [(0, 492, 20), (20, 0, 512), (532, 0, 300)]:
                    lo = max(r0, su)
                    hi = min(r0 + nr, su + cnt_u)
                    if lo >= hi:
                        continue
                    u0 = du + (lo - su)
                    for (dv, sv, cnt_v) in [(0, 492, 20), (20, 0, 512), (532, 0, 300)]:
                        nc.sync.dma_start(
                            dram_ap(ET2, u0 * WET + dv, [[WET, hi - lo], [1, cnt_v]]),
                            t[lo - r0:hi - r0, sv:sv + cnt_v])

            # gather helper: (a,b) map from ET2: val = Dsum[(a-b)%512,(a+b+delta)%512]
            # ET2[u,v]=Dsum[(v-20)%512,(u-20)%512] -> u = a+b+delta+20, v = a-b+532  (532%512==20)
            def gather_ab(delta, a0, b0, nr, ncol, dst):
                u0 = a0 + b0 + delta + 20
                v0 = a0 - b0 + 532
                nc.sync.dma_start(dst, dram_ap(ET2, u0 * WET + v0,
                                               [[WET + 1, nr], [WET - 1, ncol]]))

            # ================= stage 5: S1 = conv_f_qper(P1s, shift11) =================
            # padded P1s: rows a=k-6 in [-6,261), cols b=c-6 in [-6,517)
            P1p = []
            for k0 in range(0, 267, 128):
                kk = min(128, 267 - k0)
                t = work.tile([kk, 523], FP, tag=f"P1p_{k0}")
                gather_ab(1, k0 - 6, -6, kk, 523, t[:])
                P1p.append((k0, kk, t))
            S1t = conv2(P1p, 267, 523, "Bf256", "Bf512", HH, W, keep, "S1")
            # E1 = vstack(rot256(S1), S1)
            for (r0, nr, t) in S1t:
                nc.sync.dma_start(E1.ap()[r0:r0 + nr, 0:256], t[:, 256:512])
                nc.sync.dma_start(E1.ap()[r0:r0 + nr, 256:512], t[:, 0:256])
                nc.sync.dma_start(E1.ap()[256 + r0:256 + r0 + nr, :], t[:])

            # X0s = (P0s - 8*S1)/sqrt2
            X0t = []
            for (r0, nr, s1) in S1t:
                p0 = work.tile([nr, W], FP, tag=f"P0g_{r0}")
                gather_ab(0, r0, 0, nr, W, p0[:])
                x0 = keep.tile([nr, W], FP, tag=f"X0_{r0}")
                nc.vector.scalar_tensor_tensor(x0[:], s1[:], -8.0, p0[:],
                                               mybir.AluOpType.mult, mybir.AluOpType.add)
                nc.vector.tensor_scalar_mul(x0[:], x0[:], 1.0 / SQ2)
                nc.sync.dma_start(X0d.ap()[r0:r0 + nr, :], x0[:])
                X0t.append((r0, nr, x0))

            # ================= stage 6: S2 = conv_f_qper(X0s, shift00) =================
            X0p = pad_per_from_dram(X0d, HH, W, 5, 5, 267, 523, work, "X0p", qper=True)
            S2t = conv2(X0p, 267, 523, "Bf256", "Bf512", HH, W, keep, "S2")
            for (r0, nr, t) in S2t:
                nc.sync.dma_start(E2.ap()[r0:r0 + nr, 0:256], t[:, 256:512])
                nc.sync.dma_start(E2.ap()[r0:r0 + nr, 256:512], t[:, 0:256])
                nc.sync.dma_start(E2.ap()[256 + r0:256 + r0 + nr, :], t[:])

            # XXs = X0s - sqrt2*P1s - 8*S2 ; EXX[p,q]=XXs[(p-16)%256, ((q-280)+256*floor((p-16)/256))%512]
            for (r0, nr, s2) in S2t:
                p1 = work.tile([nr, W], FP, tag=f"P1g_{r0}")
                gather_ab(1, r0, 0, nr, W, p1[:])
                xx = work.tile([nr, W], FP, tag=f"XX_{r0}")
                x0 = X0t[r0 // 128][2]
                nc.vector.scalar_tensor_tensor(xx[:], s2[:], -8.0, x0[:],
                                               mybir.AluOpType.mult, mybir.AluOpType.add)
                nc.vector.scalar_tensor_tensor(xx[:], p1[:], -SQ2, xx[:],
                                               mybir.AluOpType.mult, mybir.AluOpType.add)
                # strips: p in [4,16): k=-1 rot 256+280; [16,272): k=0; [272,528): k=1; [528,540): k=2
                for (p0s, sr0, cnt, k) in [(4, 244, 12, -1), (16, 0, 256, 0), (272, 0, 256, 1), (528, 0, 12, 2)]:
                    lo = max(sr0, r0)
                    hi = min(sr0 + cnt, r0 + nr)
                    if lo >= hi:
                        continue
                    pp = p0s + (lo - sr0)
                    rot = (256 * k - 280) % W
                    c = 0
                    while c < WET:
                        sc = (c + rot) % W
                        seg = min(WET - c, W - sc)
                        nc.sync.dma_start(dram_ap(EXX, pp * WET + c, [[WET, hi - lo], [1, seg]]),
                                          xx[lo - r0:hi - r0, sc:sc + seg])
                        c += seg

            # ================= stage 7: S3 = conv_f_per(P1Cs, shift11) =================
            # P1Cs_pad[k,c]: (i,j)=(k-6,c-6), A=i+j+1, B=j-i: EXX row A+16, col B+280
            def gather_ij(hd, wid, Aoff, roff, coff, i0, j0, nr, ncol, dst):
                # row = (i+j+Aoff)+roff, col = (j-i)+coff
                p0 = i0 + j0 + Aoff + roff
                q0 = j0 - i0 + coff
                nc.sync.dma_start(dst, dram_ap(hd, p0 * wid + q0,
                                               [[wid - 1, nr], [wid + 1, ncol]]))

            P1Cp = []
            for k0 in range(0, 267, 128):
                kk = min(128, 267 - k0)
                t = work.tile([kk, 267], FP, tag=f"P1Cp_{k0}")
                gather_ij(EXX, WET, 1, 16, 280, k0 - 6, -6, kk, 267, t[:])
                P1Cp.append((k0, kk, t))
            S3t = conv2(P1Cp, 267, 267, "Bf256", "Bf256", HH, HH, keep, "S3")

            # E0s = (P0Bs - 16*S3)/sqrt2
            for (r0, nr, s3) in S3t:
                pb = work.tile([nr, HH], FP, tag=f"P0B_{r0}")
                gather_ij(EXX, WET, 0, 16, 280, r0, 0, nr, HH, pb[:])
                e0 = work.tile([nr, HH], FP, tag=f"E0_{r0}")
                nc.vector.scalar_tensor_tensor(e0[:], s3[:], -16.0, pb[:],
                                               mybir.AluOpType.mult, mybir.AluOpType.add)
                nc.vector.tensor_scalar_mul(e0[:], e0[:], 1.0 / SQ2)
                nc.sync.dma_start(E0d.ap()[r0:r0 + nr, :], e0[:])

            # ================= stage 8: S4 = conv_f_per(E0s, shift00) =================
            E0p = pad_per_from_dram(E0d, HH, HH, 5, 5, 267, 267, work, "E0p")
            S4t = conv2(E0p, 267, 267, "Bf256", "Bf256", HH, HH, keep, "S4")

            # ================= stage 9: output maps =================
            # Amap=-T1e/2-S3/sq2-MEE/2 ; Bmap=T1o-S4+MOO ; Cmap=-T2e/sq2-S3/sq2+MEO ; Emap=sq2*T2o-S4-2*MOE
            Gt = {k: [] for k in ["A", "B", "C", "E"]}
            for ti, (r0, nr, s3) in enumerate(S3t):
                s4 = S4t[ti][2]
                tm = {}
                for nm, hd, Aoff in [("T1e", E1, 0), ("T1o", E1, 1), ("T2e", E2, 0), ("T2o", E2, 1)]:
                    t = work.tile([nr, HH], FP, tag=f"Tm_{nm}")
                    gather_ij(hd, 512, Aoff, 0, 256, r0, 0, nr, HH, t[:])
                    tm[nm] = t
                mp = {}
                for nm, pr, pc in [("MEE", 0, 0), ("MOO", 1, 1), ("MEO", 0, 1), ("MOE", 1, 2)]:
                    t = work.tile([nr, HH], FP, tag=f"Mp_{nm}")
                    if pc < 2:
                        nc.sync.dma_start(t[:], dram_ap(Md, (2 * r0 + pr) * W + pc, [[2 * W, nr], [2, HH]]))
                    else:
                        nc.sync.dma_start(t[:, 0:HH - 1], dram_ap(Md, (2 * r0 + pr) * W + pc, [[2 * W, nr], [2, HH - 1]]))
                        nc.sync.dma_start(t[:, HH - 1:HH], dram_ap(Md, (2 * r0 + pr) * W, [[2 * W, nr], [1, 1]]))
                    mp[nm] = t
                ga = keep.tile([nr, HH], FP, tag=f"GA_{r0}")
                nc.vector.tensor_scalar_mul(ga[:], s3[:], -1.0 / SQ2)
                nc.vector.scalar_tensor_tensor(ga[:], tm["T1e"][:], -0.5, ga[:], mybir.AluOpType.mult, mybir.AluOpType.add)
                nc.vector.scalar_tensor_tensor(ga[:], mp["MEE"][:], -0.5, ga[:], mybir.AluOpType.mult, mybir.AluOpType.add)
                gb = keep.tile([nr, HH], FP, tag=f"GB_{r0}")
                nc.vector.scalar_tensor_tensor(gb[:], s4[:], -1.0, tm["T1o"][:],
                                               mybir.AluOpType.mult, mybir.AluOpType.add)
                nc.vector.scalar_tensor_tensor(gb[:], mp["MOO"][:], 1.0, gb[:], mybir.AluOpType.mult, mybir.AluOpType.add)
                gc = keep.tile([nr, HH], FP, tag=f"GC_{r0}")
                nc.vector.scalar_tensor_tensor(gc[:], tm["T2e"][:], -1.0 / SQ2, mp["MEO"][:],
                                               mybir.AluOpType.mult, mybir.AluOpType.add)
                nc.vector.scalar_tensor_tensor(gc[:], s3[:], -1.0 / SQ2, gc[:], mybir.AluOpType.mult, mybir.AluOpType.add)
                ge = keep.tile([nr, HH], FP, tag=f"GE_{r0}")
                nc.vector.tensor_scalar_mul(ge[:], s4[:], -1.0)
                nc.vector.scalar_tensor_tensor(ge[:], tm["T2o"][:], SQ2, ge[:], mybir.AluOpType.mult, mybir.AluOpType.add)
                nc.vector.scalar_tensor_tensor(ge[:], mp["MOE"][:], -2.0, ge[:], mybir.AluOpType.mult, mybir.AluOpType.add)
                Gt["A"].append((r0, nr, ga)); Gt["B"].append((r0, nr, gb))
                Gt["C"].append((r0, nr, gc)); Gt["E"].append((r0, nr, ge))

            # ================= stage 10: per-channel outputs =================
            # e0lo = xEE/2 + A ; e1lo = -xOO + B ; e0hi = -xEO + C ; e1hi = 2*xOE + E
            specs = [("out_e0lo", 0, 0, 0.5, "A"), ("out_e1lo", 1, 1, -1.0, "B"),
                     ("out_e0hi", 0, 1, -1.0, "C"), ("out_e1hi", 1, 2, 2.0, "E")]
            for ch in range(NCH):
                for (onm, pr, pc, s, gk) in specs:
                    for (r0, nr, g) in Gt[gk]:
                        xt = work.tile([nr, HH], FP, tag=f"xg_{r0}")
                        if pc < 2:
                            nc.sync.dma_start(xt[:], dram_ap(
                                x_h, ch * H * W + (2 * r0 + pr) * W + pc, [[2 * W, nr], [2, HH]]))
                        else:
                            nc.sync.dma_start(xt[:, 0:HH - 1], dram_ap(
                                x_h, ch * H * W + (2 * r0 + pr) * W + pc, [[2 * W, nr], [2, HH - 1]]))
                            nc.sync.dma_start(xt[:, HH - 1:HH], dram_ap(
                                x_h, ch * H * W + (2 * r0 + pr) * W, [[2 * W, nr], [1, 1]]))
                        ot = work.tile([nr, HH], FP, tag=f"og_{r0}")
                        nc.vector.scalar_tensor_tensor(ot[:], xt[:], s, g[:],
                                                 mybir.AluOpType.mult, mybir.AluOpType.add)
                        nc.sync.dma_start(outs[onm].ap()[ch, r0:r0 + nr, :], ot[:])

    nc.compile()
    return nc


def kernel(x, h, g, f):
    import numpy as np
    from concourse import bass_utils
    if "nc" not in _cache:
        _cache["nc"] = _build_nc()
    nc = _cache["nc"]
    hn, gn, fn = _filters()
    mats = _host_mats(np.asarray(h, np.float32), np.asarray(g, np.float32), np.asarray(f, np.float32))
    x = np.ascontiguousarray(np.asarray(x, np.float32))
    in_maps = []
    for i in range(NCORES):
        m = {"x": x[i]}
        m.update(mats)
        in_maps.append(m)
    res = bass_utils.run_bass_kernel_spmd(nc, in_maps, core_ids=list(range(NCORES)))
    def stack(nm):
        return np.stack([res.results[i][nm] for i in range(NCORES)], axis=0)
    return (stack("out_c"), stack("out_e1lo"), stack("out_e0lo"),
            stack("out_e1hi"), stack("out_e0hi"))


# revision 6
# speedup vs baseline: 3194.0051x; 3194.0051x over previous
import numpy as np

SQ2 = 2.0 ** 0.5
H = W = 512
HH = 256  # H//2
NCH = 8
NCORES = 8

_cache = {}


def _filters():
    hh = np.array([0.037828455506995, -0.02384946501938, -0.11062440441842, 0.37740285561265], np.float32)
    h = np.concatenate([hh, [np.float32(0.8526986790094)], hh[::-1]]).astype(np.float32)
    gg = np.array([-0.064538882628938, -0.040689417609558, 0.41809227322221], np.float32)
    g = np.concatenate([gg, [np.float32(0.78848561640566)], gg[::-1]]).astype(np.float32)
    v = np.array([0.63, -0.193, 0.0972, -0.0526, 0.0272, -0.0144], np.float32)
    f = np.concatenate([v[::-1], v]).astype(np.float32)
    f[::2] = -f[::2]
    return h, g, f


def _host_mats(h, g, f):
    # BhP [520,256]: fused 9-tap h conv + 2x avgpool (rows): out[i] = 0.5*(L[2i]+L[2i+1])
    BhP = np.zeros((520, 256), np.float32)
    for i in range(256):
        for p in (0, 1):
            r = 2 * i + p
            for u in range(9):
                BhP[r + u, i] += 0.5 * h[u]
    # GU [260,512]: upsample cA (pad2) + 7-tap g conv: Mrow[r] = sum_u g[u] D[r+u-3], D[2a']=cA[a']
    GU = np.zeros((260, 512), np.float32)
    for r in range(512):
        for u in range(7):
            al = r + u - 3
            if al % 2 == 0:
                a = al // 2 + 2  # cAp row (pad 2)
                GU[a, r] += g[u]
    GU8 = (8.0 * GU).astype(np.float32)
    # Bf256 [267,256], Bf512 [523,512]: 12-tap f bands
    Bf256 = np.zeros((267, 256), np.float32)
    for o in range(256):
        for u in range(12):
            Bf256[o + u, o] = f[u]
    Bf512 = np.zeros((523, 512), np.float32)
    for o in range(512):
        for u in range(12):
            Bf512[o + u, o] = f[u]
    ident = np.eye(128, dtype=np.float32)
    return {"BhP": BhP, "GU8": GU8, "GU": GU, "Bf256": Bf256, "Bf512": Bf512, "ident": ident}


def _build_nc():
    import concourse.bass as bass
    import concourse.bacc as bacc
    import concourse.mybir as mybir
    from concourse import tile

    FP = mybir.dt.float32
    nc = bacc.Bacc("TRN2", target_bir_lowering=False, debug=False, num_devices=NCORES)
    AP = bass.AP

    x_h = nc.dram_tensor("x", [NCH, H, W], FP, kind="ExternalInput")
    mat_hs = {}
    for nm, shp in [("BhP", (520, 256)), ("GU8", (260, 512)), ("GU", (260, 512)),
                    ("Bf256", (267, 256)), ("Bf512", (523, 512)), ("ident", (128, 128))]:
        mat_hs[nm] = nc.dram_tensor(nm, list(shp), FP, kind="ExternalInput")
    outs = {}
    for nm in ["out_c", "out_e1lo", "out_e0lo", "out_e1hi", "out_e0hi"]:
        outs[nm] = nc.dram_tensor(nm, [NCH, HH, HH], FP, kind="ExternalOutput")

    # internal DRAM
    WET = 832
    Xd = nc.dram_tensor("Xd", [H, W], FP, kind="Internal")
    cAd = nc.dram_tensor("cAd", [HH, HH], FP, kind="Internal")
    Md = nc.dram_tensor("Md", [H, W], FP, kind="Internal")
    ET2 = nc.dram_tensor("ET2", [WET, WET], FP, kind="Internal")       # DsumT tiled, origin 20,20
    EXX = nc.dram_tensor("EXX", [560, WET], FP, kind="Internal")       # XXs qext, row A+16, col B+280
    E1 = nc.dram_tensor("E1", [512, 512], FP, kind="Internal")         # S1 qext: vstack(rot256(S1), S1)
    E2 = nc.dram_tensor("E2", [512, 512], FP, kind="Internal")
    X0d = nc.dram_tensor("X0d", [HH, W], FP, kind="Internal")
    E0d = nc.dram_tensor("E0d", [HH, HH], FP, kind="Internal")

    def dram_ap(hd, off, dims):
        return AP(hd, off, [list(d) for d in dims])

    with tile.TileContext(nc) as tc:
        import contextlib
        ctx = contextlib.ExitStack()
        with ctx:
            cpool = ctx.enter_context(tc.tile_pool(name="consts", bufs=1))
            work = ctx.enter_context(tc.tile_pool(name="work", bufs=2))
            keep = ctx.enter_context(tc.tile_pool(name="keep", bufs=1))
            tmp = ctx.enter_context(tc.tile_pool(name="tmp", bufs=1))
            outp = ctx.enter_context(tc.tile_pool(name="outp", bufs=4))
            psum = ctx.enter_context(tc.tile_pool(name="ps", bufs=4, space="PSUM"))

            # ---- load const matrices as K-chunked [<=128, M] tiles ----
            mats = {}
            for nm, (K, M) in [("BhP", (520, 256)), ("GU8", (260, 512)), ("GU", (260, 512)),
                               ("Bf256", (267, 256)), ("Bf512", (523, 512))]:
                tl = []
                for k0 in range(0, K, 128):
                    kk = min(128, K - k0)
                    t = cpool.tile([kk, M], FP, tag=f"m_{nm}_{k0}")
                    nc.sync.dma_start(t[:], mat_hs[nm].ap()[k0:k0 + kk, :])
                    tl.append((k0, kk, t))
                mats[nm] = tl
            identt = cpool.tile([128, 128], FP, tag="ident")
            nc.sync.dma_start(identt[:], mat_hs["ident"].ap()[:, :])

            def transpose_tiles(src_tiles, R, C, pool, tag):
                # src_tiles: list (r0, nr, tile[nr, C]) covering [R, C] -> returns tiles of [C, R]
                outt = []
                for c0 in range(0, C, 128):
                    cw = min(128, C - c0)
                    t = pool.tile([cw, R], FP, tag=f"{tag}_{c0}")
                    for (r0, nr, st) in src_tiles:
                        ps = psum.tile([cw, nr], FP, tag="ps")
                        nc.tensor.transpose(ps[:, :], st[:, c0:c0 + cw], identt[:nr, :nr])
                        nc.vector.tensor_copy(t[:, r0:r0 + nr], ps[:, :])
                    outt.append((c0, cw, t))
                return outt

            def band_pass(in_tiles, mat_tiles, M, N, pool, tag):
                # out[m, n] = sum_k mat[k, m] * in[k, n]; in_tiles chunked at 128 rows
                outt = []
                for m0 in range(0, M, 128):
                    mw = min(128, M - m0)
                    t = pool.tile([mw, N], FP, tag=f"{tag}_{m0}")
                    for n0 in range(0, N, 512):
                        nw = min(512, N - n0)
                        ps = psum.tile([mw, nw], FP, tag="ps")
                        nk = len(in_tiles)
                        for ki, ((k0, kk, it), (mk0, mkk, mt)) in enumerate(zip(in_tiles, mat_tiles)):
                            assert k0 == mk0 and kk == mkk
                            nc.tensor.matmul(ps[:, :], mt[:, m0:m0 + mw], it[:, n0:n0 + nw],
                                             start=(ki == 0), stop=(ki == nk - 1))
                        nc.vector.tensor_copy(t[:, n0:n0 + nw], ps[:, :])
                    outt.append((m0, mw, t))
                return outt

            def conv2(in_tiles, Rp, Cp, mrow, mcol, Mr, Mc, pool, tag):
                # in_tiles cover padded [Rp, Cp]; returns result [Mr, Mc] tiles (normal orientation)
                p1 = band_pass(in_tiles, mats[mrow], Mr, Cp, tmp, "cvp1")
                p1t = transpose_tiles(p1, Mr, Cp, tmp, "cvt1")
                p2 = band_pass(p1t, mats[mcol], Mc, Mr, tmp, "cvp2")  # [Mc, Mr]
                return transpose_tiles(p2, Mc, Mr, pool, tag)

            def pad_per_from_dram(hd, R, C, ru, cl, Rp, Cp, pool, tag, qper=False):
                # build padded tiles [Rp, Cp] from DRAM map [R, C]; padded (k,c) -> src row (k-ru)%R col (c-cl)%C
                # qper: rows outside [0,R) additionally rotate cols by C//2
                tiles = []
                for k0 in range(0, Rp, 128):
                    kk = min(128, Rp - k0)
                    t = tmp.tile([kk, Cp], FP, tag=f"pad_{k0}")
                    # split rows into runs with same wrap-band
                    k = k0
                    while k < k0 + kk:
                        a = k - ru  # source row index (unwrapped)
                        band = 0 if 0 <= a < R else (-1 if a < 0 else 1)
                        # run length until band changes or tile ends
                        if band == -1:
                            run = min(k0 + kk - k, -a)
                        elif band == 0:
                            run = min(k0 + kk - k, R - a)
                        else:
                            run = k0 + kk - k
                        sr = a % R
                        rot = (C // 2) if (qper and band != 0) else 0
                        # cols: padded c -> src (c - cl + rot) % C ; emit contiguous segments
                        c = 0
                        while c < Cp:
                            sc = (c - cl + rot) % C
                            seg = min(Cp - c, C - sc)
                            nc.sync.dma_start(
                                t[k - k0:k - k0 + run, c:c + seg],
                                dram_ap(hd, sr * C + sc, [[C, run], [1, seg]]))
                            c += seg
                        k += run
                    tiles.append((k0, kk, t))
                return tiles

            # ================= stage 1: X = channel sum of x =================
            for r0 in range(0, H, 128):
                xa = work.tile([128, W], FP, tag="xsumA")
                xb = work.tile([128, W], FP, tag="xsumB")
                acc = work.tile([128, W], FP, tag="xsumAcc")
                nc.sync.dma_start(xa[:], x_h.ap()[0, r0:r0 + 128, :])
                nc.sync.dma_start(xb[:], x_h.ap()[1, r0:r0 + 128, :])
                nc.vector.scalar_tensor_tensor(acc[:], xa[:], 1.0, xb[:],
                                               mybir.AluOpType.mult, mybir.AluOpType.add)
                for ch in range(2, NCH):
                    xc = work.tile([128, W], FP, tag="xsumC")
                    nc.sync.dma_start(xc[:], x_h.ap()[ch, r0:r0 + 128, :])
                    nc.vector.scalar_tensor_tensor(acc[:], xc[:], 1.0, acc[:],
                                                   mybir.AluOpType.mult, mybir.AluOpType.add)
                nc.sync.dma_start(Xd.ap()[r0:r0 + 128, :], acc[:])

            # ================= stage 2: cA = pool(conv_h(X)) =================
            Xp = pad_per_from_dram(Xd, H, W, 4, 4, 520, 520, work, "Xp")
            cAt = conv2(Xp, 520, 520, "BhP", "BhP", 256, 256, keep, "cA")
            for (r0, nr, t) in cAt:
                nc.sync.dma_start(cAd.ap()[r0:r0 + nr, :], t[:])
                for ch in range(NCH):
                    nc.sync.dma_start(outs["out_c"].ap()[ch, r0:r0 + nr, :], t[:])

            # ================= stage 3: M = 8*conv_g(dup(cA)) =================
            cAp = pad_per_from_dram(cAd, HH, HH, 2, 2, 260, 260, work, "cAp")
            Mt = conv2(cAp, 260, 260, "GU8", "GU", H, W, keep, "M")
            for (r0, nr, t) in Mt:
                nc.sync.dma_start(Md.ap()[r0:r0 + nr, :], t[:])

            # ================= stage 4: Dsum = X - 8M; DsumT -> ET2 =================
            Dst = []
            for (r0, nr, mt) in Mt:
                xt = work.tile([128, W], FP, tag="Xrd")
                nc.sync.dma_start(xt[:], Xd.ap()[r0:r0 + nr, :])
                d = tmp.tile([128, W], FP, tag=f"Ds_{r0}")
                nc.vector.scalar_tensor_tensor(d[:], mt[:], -8.0, xt[:],
                                               mybir.AluOpType.mult, mybir.AluOpType.add)
                Dst.append((r0, nr, d))
            DsT = transpose_tiles(Dst, H, W, tmp, "DsT")
            # ET2[u,v] = DsumT[(u-20)%512, (v-20)%512], [832,832]
            for (r0, nr, t) in DsT:
                for (du, su, cnt_u) in [(0, 492, 20), (20, 0, 512), (532, 0, 300)]:
                    lo = max(r0, su)
                    hi = min(r0 + nr, su + cnt_u)
                    if lo >= hi:
                        continue
                    u0 = du + (lo - su)
                    for (dv, sv, cnt_v) in [(0, 492, 20), (20, 0, 512), (532, 0, 300)]:
                        nc.sync.dma_start(
                            dram_ap(ET2, u0 * WET + dv, [[WET, hi - lo], [1, cnt_v]]),
                            t[lo - r0:hi - r0, sv:sv + cnt_v])

            # gather helper: (a,b) map from ET2: val = Dsum[(a-b)%512,(a+b+delta)%512]
            # ET2[u,v]=Dsum[(v-20)%512,(u-20)%512] -> u = a+b+delta+20, v = a-b+532  (532%512==20)
            def gather_ab(delta, a0, b0, nr, ncol, dst):
                u0 = a0 + b0 + delta + 20
                v0 = a0 - b0 + 532
                nc.sync.dma_start(dst, dram_ap(ET2, u0 * WET + v0,
                                               [[WET + 1, nr], [WET - 1, ncol]]))

            # ================= stage 5: S1 = conv_f_qper(P1s, shift11) =================
            # padded P1s: rows a=k-6 in [-6,261), cols b=c-6 in [-6,517)
            P1p = []
            for k0 in range(0, 267, 128):
                kk = min(128, 267 - k0)
                t = work.tile([kk, 523], FP, tag=f"P1p_{k0}")
                gather_ab(1, k0 - 6, -6, kk, 523, t[:])
                P1p.append((k0, kk, t))
            S1t = conv2(P1p, 267, 523, "Bf256", "Bf512", HH, W, keep, "S1")
            # E1 = vstack(rot256(S1), S1)
            for (r0, nr, t) in S1t:
                nc.sync.dma_start(E1.ap()[r0:r0 + nr, 0:256], t[:, 256:512])
                nc.sync.dma_start(E1.ap()[r0:r0 + nr, 256:512], t[:, 0:256])
                nc.sync.dma_start(E1.ap()[256 + r0:256 + r0 + nr, :], t[:])

            # X0s = (P0s - 8*S1)/sqrt2
            X0t = []
            for (r0, nr, s1) in S1t:
                p0 = work.tile([nr, W], FP, tag=f"P0g_{r0}")
                gather_ab(0, r0, 0, nr, W, p0[:])
                x0 = keep.tile([nr, W], FP, tag=f"X0_{r0}")
                nc.vector.scalar_tensor_tensor(x0[:], s1[:], -8.0, p0[:],
                                               mybir.AluOpType.mult, mybir.AluOpType.add)
                nc.vector.tensor_scalar_mul(x0[:], x0[:], 1.0 / SQ2)
                nc.sync.dma_start(X0d.ap()[r0:r0 + nr, :], x0[:])
                X0t.append((r0, nr, x0))

            # ================= stage 6: S2 = conv_f_qper(X0s, shift00) =================
            X0p = pad_per_from_dram(X0d, HH, W, 5, 5, 267, 523, work, "X0p", qper=True)
            S2t = conv2(X0p, 267, 523, "Bf256", "Bf512", HH, W, keep, "S2")
            for (r0, nr, t) in S2t:
                nc.sync.dma_start(E2.ap()[r0:r0 + nr, 0:256], t[:, 256:512])
                nc.sync.dma_start(E2.ap()[r0:r0 + nr, 256:512], t[:, 0:256])
                nc.sync.dma_start(E2.ap()[256 + r0:256 + r0 + nr, :], t[:])

            # XXs = X0s - sqrt2*P1s - 8*S2 ; EXX[p,q]=XXs[(p-16)%256, ((q-280)+256*floor((p-16)/256))%512]
            for (r0, nr, s2) in S2t:
                p1 = work.tile([nr, W], FP, tag=f"P1g_{r0}")
                gather_ab(1, r0, 0, nr, W, p1[:])
                xx = work.tile([nr, W], FP, tag=f"XX_{r0}")
                x0 = X0t[r0 // 128][2]
                nc.vector.scalar_tensor_tensor(xx[:], s2[:], -8.0, x0[:],
                                               mybir.AluOpType.mult, mybir.AluOpType.add)
                nc.vector.scalar_tensor_tensor(xx[:], p1[:], -SQ2, xx[:],
                                               mybir.AluOpType.mult, mybir.AluOpType.add)
                # strips: p in [4,16): k=-1 rot 256+280; [16,272): k=0; [272,528): k=1; [528,540): k=2
                for (p0s, sr0, cnt, k) in [(4, 244, 12, -1), (16, 0, 256, 0), (272, 0, 256, 1), (528, 0, 12, 2)]:
                    lo = max(sr0, r0)
                    hi = min(sr0 + cnt, r0 + nr)
                    if lo >= hi:
                        continue
                    pp = p0s + (lo - sr0)
                    rot = (256 * k - 280) % W
                    c = 0
                    while c < WET:
                        sc = (c + rot) % W
                        seg = min(WET - c, W - sc)
                        nc.sync.dma_start(dram_ap(EXX, pp * WET + c, [[WET, hi - lo], [1, seg]]),
                                          xx[lo - r0:hi - r0, sc:sc + seg])
                        c += seg

            # ================= stage 7: S3 = conv_f_per(P1Cs, shift11) =================
            # P1Cs_pad[k,c]: (i,j)=(k-6,c-6), A=i+j+1, B=j-i: EXX row A+16, col B+280
            def gather_ij(hd, wid, Aoff, roff, coff, i0, j0, nr, ncol, dst):
                # row = (i+j+Aoff)+roff, col = (j-i)+coff
                p0 = i0 + j0 + Aoff + roff
                q0 = j0 - i0 + coff
                nc.sync.dma_start(dst, dram_ap(hd, p0 * wid + q0,
                                               [[wid - 1, nr], [wid + 1, ncol]]))

            P1Cp = []
            for k0 in range(0, 267, 128):
                kk = min(128, 267 - k0)
                t = work.tile([kk, 267], FP, tag=f"P1Cp_{k0}")
                gather_ij(EXX, WET, 1, 16, 280, k0 - 6, -6, kk, 267, t[:])
                P1Cp.append((k0, kk, t))
            S3t = conv2(P1Cp, 267, 267, "Bf256", "Bf256", HH, HH, keep, "S3")

            # E0s = (P0Bs - 16*S3)/sqrt2
            for (r0, nr, s3) in S3t:
                pb = work.tile([nr, HH], FP, tag=f"P0B_{r0}")
                gather_ij(EXX, WET, 0, 16, 280, r0, 0, nr, HH, pb[:])
                e0 = work.tile([nr, HH], FP, tag=f"E0_{r0}")
                nc.vector.scalar_tensor_tensor(e0[:], s3[:], -16.0, pb[:],
                                               mybir.AluOpType.mult, mybir.AluOpType.add)
                nc.vector.tensor_scalar_mul(e0[:], e0[:], 1.0 / SQ2)
                nc.sync.dma_start(E0d.ap()[r0:r0 + nr, :], e0[:])

            # ================= stage 8: S4 = conv_f_per(E0s, shift00) =================
            E0p = pad_per_from_dram(E0d, HH, HH, 5, 5, 267, 267, work, "E0p")
            S4t = conv2(E0p, 267, 267, "Bf256", "Bf256", HH, HH, keep, "S4")

            # ================= stage 9: output maps =================
            # Amap=-T1e/2-S3/sq2-MEE/2 ; Bmap=T1o-S4+MOO ; Cmap=-T2e/sq2-S3/sq2+MEO ; Emap=sq2*T2o-S4-2*MOE
            Gt = {k: [] for k in ["A", "B", "C", "E"]}
            for ti, (r0, nr, s3) in enumerate(S3t):
                s4 = S4t[ti][2]
                tm = {}
                for nm, hd, Aoff in [("T1e", E1, 0), ("T1o", E1, 1), ("T2e", E2, 0), ("T2o", E2, 1)]:
                    t = work.tile([nr, HH], FP, tag=f"Tm_{nm}")
                    gather_ij(hd, 512, Aoff, 0, 256, r0, 0, nr, HH, t[:])
                    tm[nm] = t
                mp = {}
                for nm, pr, pc in [("MEE", 0, 0), ("MOO", 1, 1), ("MEO", 0, 1), ("MOE", 1, 2)]:
                    t = work.tile([nr, HH], FP, tag=f"Mp_{nm}")
                    if pc < 2:
                        nc.sync.dma_start(t[:], dram_ap(Md, (2 * r0 + pr) * W + pc, [[2 * W, nr], [2, HH]]))
                    else:
                        nc.sync.dma_start(t[:, 0:HH - 1], dram_ap(Md, (2 * r0 + pr) * W + pc, [[2 * W, nr], [2, HH - 1]]))
                        nc.sync.dma_start(t[:, HH - 1:HH], dram_ap(Md, (2 * r0 + pr) * W, [[2 * W, nr], [1, 1]]))
                    mp[nm] = t
                ga = keep.tile([nr, HH], FP, tag=f"GA_{r0}")
                nc.vector.tensor_scalar_mul(ga[:], s3[:], -1.0 / SQ2)
                nc.vector.scalar_tensor_tensor(ga[:], tm["T1e"][:], -0.5, ga[:], mybir.AluOpType.mult, mybir.AluOpType.add)
                nc.vector.scalar_tensor_tensor(ga[:], mp["MEE"][:], -0.5, ga[:], mybir.AluOpType.mult, mybir.AluOpType.add)
                gb = keep.tile([nr, HH], FP, tag=f"GB_{r0}")
                nc.vector.scalar_tensor_tensor(gb[:], s4[:], -1.0, tm["T1o"][:],
                                               mybir.AluOpType.mult, mybir.AluOpType.add)
                nc.vector.scalar_tensor_tensor(gb[:], mp["MOO"][:], 1.0, gb[:], mybir.AluOpType.mult, mybir.AluOpType.add)
                gc = keep.tile([nr, HH], FP, tag=f"GC_{r0}")
                nc.vector.scalar_tensor_tensor(gc[:], tm["T2e"][:], -1.0 / SQ2, mp["MEO"][:],
                                               mybir.AluOpType.mult, mybir.AluOpType.add)
                nc.vector.scalar_tensor_tensor(gc[:], s3[:], -1.0 / SQ2, gc[:], mybir.AluOpType.mult, mybir.AluOpType.add)
                ge = keep.tile([nr, HH], FP, tag=f"GE_{r0}")
                nc.vector.tensor_scalar_mul(ge[:], s4[:], -1.0)
                nc.vector.scalar_tensor_tensor(ge[:], tm["T2o"][:], SQ2, ge[:], mybir.AluOpType.mult, mybir.AluOpType.add)
                nc.vector.scalar_tensor_tensor(ge[:], mp["MOE"][:], -2.0, ge[:], mybir.AluOpType.mult, mybir.AluOpType.add)
                Gt["A"].append((r0, nr, ga)); Gt["B"].append((r0, nr, gb))
                Gt["C"].append((r0, nr, gc)); Gt["E"].append((r0, nr, ge))

            # ================= stage 10: per-channel outputs =================
            # e0lo = xEE/2 + A ; e1lo = -xOO + B ; e0hi = -xEO + C ; e1hi = 2*xOE + E
            specs = [("out_e0lo", 0, 0, 0.5, "A"), ("out_e1lo", 1, 1, -1.0, "B"),
                     ("out_e0hi", 0, 1, -1.0, "C"), ("out_e1hi", 1, 2, 2.0, "E")]
            for ch in range(NCH):
                for (onm, pr, pc, s, gk) in specs:
                    for (r0, nr, g) in Gt[gk]:
                        xt = outp.tile([nr, HH], FP, tag="xg")
                        if pc < 2:
                            nc.sync.dma_start(xt[:], dram_ap(
                                x_h, ch * H * W + (2 * r0 + pr) * W + pc, [[2 * W, nr], [2, HH]]))
                        else:
                            nc.sync.dma_start(xt[:, 0:HH - 1], dram_ap(
                                x_h, ch * H * W + (2 * r0 + pr) * W + pc, [[2 * W, nr], [2, HH - 1]]))
                            nc.sync.dma_start(xt[:, HH - 1:HH], dram_ap(
                                x_h, ch * H * W + (2 * r0 + pr) * W, [[2 * W, nr], [1, 1]]))
                        ot = outp.tile([nr, HH], FP, tag="og")
                        nc.vector.scalar_tensor_tensor(ot[:], xt[:], s, g[:],
                                                 mybir.AluOpType.mult, mybir.AluOpType.add)
                        nc.sync.dma_start(outs[onm].ap()[ch, r0:r0 + nr, :], ot[:])

    nc.compile()
    return nc


def kernel(x, h, g, f):
    import numpy as np
    from concourse import bass_utils
    if "nc" not in _cache:
        _cache["nc"] = _build_nc()
    nc = _cache["nc"]
    hn, gn, fn = _filters()
    mats = _host_mats(np.asarray(h, np.float32), np.asarray(g, np.float32), np.asarray(f, np.float32))
    x = np.ascontiguousarray(np.asarray(x, np.float32))
    in_maps = []
    for i in range(NCORES):
        m = {"x": x[i]}
        m.update(mats)
        in_maps.append(m)
    res = bass_utils.run_bass_kernel_spmd(nc, in_maps, core_ids=list(range(NCORES)))
    def stack(nm):
        return np.stack([res.results[i][nm] for i in range(NCORES)], axis=0)
    return (stack("out_c"), stack("out_e1lo"), stack("out_e0lo"),
            stack("out_e1hi"), stack("out_e0hi"))
